# revision 24
# baseline (speedup 1.0000x reference)
"""Trainium2 Bass kernel: single-channel 15x15 cross-correlation (pad=1,
stride=1) of a 4096x4096 fp32 image, + scalar bias.

Strategy
--------
fp8 (e4m3) matmuls in DoubleRow perf mode (K=256, 0.5 PE cycles per output
column) with the image columns folded 8-wide into the partition dim:
partition p = (row-in-block)*8 + col-phase, one k-tile block = 16 image rows.
DoubleRow pairs two adjacent blocks, so one matmul contracts a 32-row x
8-phase input window against a banded weight matrix whose M=128 output
partitions are 16 output rows x 8 col-phases.  A 15x15 kernel then needs only
3 column-shift matmuls (dj = 8t + s_in - s_out) per conv term.

fp8 alone is too coarse (rel err ~3e-2), so the conv is split into three fp8
convs: x_hi*w_hi + x_lo*w_hi + x_hi*w_lo with x = x_hi + x_lo, w = w_hi +
w_lo both split host-side into fp8 value + fp8 residual (dropping the
second-order term), giving ~1e-3 rel err.  9 matmuls of N=511 accumulate one
PSUM bank per strip of 16 output rows x 4088 output cols.

Each core owns 32 strips (512 output rows): 9*32 matmuls * 511 cols * 0.5
cycles at 2.4 GHz ~= 31 us of PE stream time.  Inputs load once (33 folded
blocks per array per core, halo included host-side, no collectives); outputs
store as folded bf16 and the host unfolds, crops, and adds the bias.
"""

import os

import numpy as np

KH = KW = 15
PAD = 1
H = W = 4096
OUT = H + 2 * PAD - KH + 1  # 4084
NCORES = 8
F = 8  # column fold factor
RB = 16  # image rows per fold block (one k-tile: RB*F = 128 partitions)
NSTRIP = 32  # strips (16 output rows each) per core
NBLK = NSTRIP + 1  # fold blocks per core (one halo block)
NFC = 513  # folded input cols per block (8*513 = 4104 padded cols)
NFO = 511  # folded output cols per strip (8*511 = 4088 >= 4084)
NT = 3  # column-shift matmuls per conv term
XROWS = RB * (NSTRIP * NCORES + 1)  # 4112 padded image rows
XCOLS = F * NFC  # 4104 padded image cols
DMA_BLKS = 3  # fold blocks per input DMA

LAST_RESULT = None  # BassKernelResults of the most recent run (for test.py)


def _patch_drain():
    """walrus's CTRL_NO instruction struct holds very few semaphore waits;
    Tile's kernel-tail drain aggregates one wait per logical processor and
    overflows it.  Spread the waits across 1-wait-per-nop SP instructions."""
    import concourse.mybir as mybir
    import concourse.tile as tile
    from concourse.vector_clock import ScopedClock

    def _split_drain_and_barrier(self, tick_clock, wait_clock):
        nc = self.nc
        probe = nc.sync.nop(nofuse=True)
        wait_clock.add_sem_waits(
            probe.ins, ScopedClock({None: tick_clock.global_clock})
        )
        si = probe.ins.sync_info
        if si is not None and len(si.on_wait) > 1:
            waits = list(si.on_wait)
            probe.ins.sync_info = mybir.SyncInfo(
                on_wait=waits[:1], on_update=list(si.on_update)
            )
            # Spread the 1-wait-per-nop chain across all engines: each nop
            # costs ~115 ns of engine queue time, and the NEFF ends when the
            # slowest engine finishes, so round-robin cuts the drain tail
            # roughly 5x versus stacking every wait on SP.
            lanes = [nc.sync, nc.scalar, nc.vector, nc.gpsimd, nc.tensor]
            for i, w in enumerate(waits[1:]):
                extra = lanes[i % len(lanes)].nop(nofuse=True)
                extra.ins.sync_info = mybir.SyncInfo(on_wait=[w], on_update=[])
        nc.sync.drain()
        # The stock exit path does barrier -> semaphore cleanup -> barrier
        # (~8us).  This NEFF executes once per load, so leftover semaphore
        # values don't matter: skip the cleanup, keep only the drain (which
        # carries the waits that guarantee all DMAs have landed).
        assert self.sems is not None
        popped = nc._tile_sem_poison_stack.pop()
        assert popped is self._sem_poison

    tile.TileContext._drain_and_barrier = _split_drain_and_barrier


def _split_multi_waits(nc):
    """This compiler's TPB instruction structs hold only one sync-wait slot
    (walrus setupSyncWait rejects more).  Tile sometimes assigns 2+ waits
    (DMA completion + slot release) to one instruction; split the excess onto
    same-engine nops inserted immediately before it."""
    import concourse.mybir as mybir

    for fn in nc.m.functions:
        for bb in fn.blocks:
            insts = list(bb.instructions)
            out = []
            changed = False
            for inst in insts:
                si = inst.sync_info
                if (
                    not isinstance(inst, mybir.InstNoOp)
                    and si is not None
                    and len(si.on_wait) > 1
                ):
                    waits = list(si.on_wait)
                    for w in waits[:-1]:
                        nop = mybir.InstNoOp(
                            name=nc.get_next_instruction_name(),
                            engine=inst.engine,
                            bass_nofuse=True,
                            sync_info=mybir.SyncInfo(on_wait=[w], on_update=[]),
                        )
                        nc.register_instruction(nop)
                        out.append(nop)
                    inst.sync_info = mybir.SyncInfo(
                        on_wait=[waits[-1]], on_update=list(si.on_update)
                    )
                    changed = True
                out.append(inst)
            if changed:
                bb.instructions = out


def _make_bands(w):
    """B[t][p, i, m] = w[di, dj] with di = 16i + p//8 - m//8,
    dj = 8t + p%8 - m%8 (zero outside the 15x15 support)."""
    B = np.zeros((NT, 128, 2, 128), np.float32)
    p = np.arange(128)
    r_, s_in = p // 8, p % 8
    m = np.arange(128)
    m_row, s_out = m // 8, m % 8
    for t in range(NT):
        for i in range(2):
            di = (RB * i + r_)[:, None] - m_row[None, :]
            dj = F * t + s_in[:, None] - s_out[None, :]
            valid = (di >= 0) & (di < KH) & (dj >= 0) & (dj < KW)
            B[t, :, i, :][valid] = w[di[valid], dj[valid]]
    return B


def _fold(arr8):
    """[XROWS, XCOLS] fp8 -> [nblocks, 128, NFC]: block g holds image rows
    [16g, 16g+16), partition p = (row%16)*8 + (col%8), free n = col//8."""
    g = XROWS // RB
    return np.ascontiguousarray(
        arr8.reshape(g, RB, NFC, F).transpose(0, 1, 3, 2).reshape(g, 128, NFC)
    )


def _build_program(bias_val):
    import concourse.bass as bass
    import concourse.mybir as mybir
    import concourse.tile as tile

    _patch_drain()
    f8 = mybir.dt.float8e4
    f32 = mybir.dt.float32
    bf16 = mybir.dt.bfloat16
    DR = mybir.MatmulPerfMode.DoubleRow

    nc = bass.Bass()
    xh = nc.declare_dram_parameter("xh", [128, NBLK * NFC], f8, isOutput=False)
    xl = nc.declare_dram_parameter("xl", [128, NBLK * NFC], f8, isOutput=False)
    bd = nc.declare_dram_parameter("bands", [128, 6 * 2 * 128], f8, isOutput=False)
    out = nc.declare_dram_parameter("out", [NSTRIP * 128, NFO], bf16, isOutput=True)
    warm_out = nc.declare_dram_parameter("warm_out", [128, 128], bf16, isOutput=True)

    with tile.TileContext(nc) as tc:
        with (
            tc.tile_pool(name="const", bufs=1) as constp,
            tc.tile_pool(name="xp", bufs=1) as xp,
            tc.tile_pool(name="psum", bufs=6, space="PSUM") as psp,
            tc.tile_pool(name="wps", bufs=1, space="PSUM") as wpsp,
            tc.tile_pool(name="op", bufs=6) as outp,
        ):
            # PE p-state warm-up: the tensor clock ramps only after ~3us of
            # sustained work, and the first input DMAs take ~3us to land.
            # Burn that window with matmuls on an uninitialized scratch tile
            # so the real matmuls start at full clock.
            wt = constp.tile([128, 256], f8, tag="warm")
            wps = wpsp.tile([128, 128], f32, tag="wps")
            nc.gpsimd.memset(wt[:, :], 0)
            # PE p-state warm-up: bridge the ~3us between the preamble barrier
            # and the first input blocks landing, so the clock is fully ramped
            # (and never idles, which resets the ramp) when real work starts.
            for k in range(34):
                nc.tensor.matmul(
                    wps[:, :], wt[:, 0:128], wt[:, 128:256], start=True, stop=True
                )

            # Per-band DMAs so the first matmul only waits on a 32 KB load.
            bt = constp.tile([128, 6, 2, 128], f8, tag="bands")
            for k in range(6):
                nc.scalar.dma_start(
                    out=bt[:, k, :, :], in_=bd[:, 256 * k : 256 * (k + 1)]
                )

            # Input blocks: x_hi rides the sync ring, x_lo the gpsimd ring.
            # The first blocks go per-block so a consuming strip never waits
            # on a multi-block chunk; later blocks batch 3-wide.  The scalar
            # ring, done with bands by ~12us, carries x_hi block 2 to ease
            # the early supply crunch on the sync ring.
            xht = xp.tile([128, NBLK, NFC], f8, tag="xh")
            xlt = xp.tile([128, NBLK, NFC], f8, tag="xl")
            nc.scalar.dma_start(out=xht[:, 2:3, :], in_=xh[:, 2 * NFC : 3 * NFC])
            chunks = [(b, b + 1) for b in range(6)] + [
                (b, min(b + DMA_BLKS, NBLK)) for b in range(6, NBLK, DMA_BLKS)
            ]
            for b0, b1 in chunks:
                if b0 != 2:
                    nc.sync.dma_start(
                        out=xht[:, b0:b1, :], in_=xh[:, b0 * NFC : b1 * NFC]
                    )
                nc.gpsimd.dma_start(
                    out=xlt[:, b0:b1, :], in_=xl[:, b0 * NFC : b1 * NFC]
                )

            # (moving tensor, band index) per conv term: x_hi*w_hi + x_lo*w_hi
            # + x_hi*w_lo; bands 0..2 hold w_hi shifts, 3..5 w_lo shifts.
            terms = ((xht, 0), (xlt, 0), (xht, 3))
            for j in range(NSTRIP):
                ps = psp.tile([128, NFO], f32, tag="ps")
                for idx, (src, sel) in enumerate(terms):
                    for t in range(NT):
                        nc.tensor.matmul(
                            ps[:, :],
                            bt[:, sel + t, :, :],
                            src[:, j : j + 2, t : t + NFO],
                            start=(idx == 0 and t == 0),
                            stop=(idx == len(terms) - 1 and t == NT - 1),
                            perf_mode=DR,
                        )
                ot = outp.tile([128, NFO], bf16, tag="ot")
                nc.vector.tensor_scalar_add(ot[:, :], ps[:, :], bias_val)
                nc.scalar.dma_start(
                    out=out[128 * j : 128 * (j + 1), :], in_=ot[:, :]
                )
                if j == 4:
                    # Drain the warm-up PSUM mid-stream, off the critical path.
                    wot = outp.tile([128, 128], bf16, tag="wot")
                    nc.vector.tensor_scalar_add(wot[:, :], wps[:, :], 0.0)
                    nc.scalar.dma_start(out=warm_out[:, :], in_=wot[:, :])

    _split_multi_waits(nc)
    return nc


def kernel(x, weight, bias):
    global LAST_RESULT
    import ml_dtypes
    from concourse.bass_utils import run_bass_kernel_spmd

    e4 = ml_dtypes.float8_e4m3
    x = np.ascontiguousarray(np.asarray(x, dtype=np.float32))
    weight = np.asarray(weight, dtype=np.float32)
    bias = np.asarray(bias, dtype=np.float32)

    xpad = np.zeros((XROWS, XCOLS), np.float32)
    xpad[PAD : PAD + H, PAD : PAD + W] = x
    x_hi = xpad.astype(e4)
    x_lo = (xpad - x_hi.astype(np.float32)).astype(e4)
    xf_hi = _fold(x_hi)
    xf_lo = _fold(x_lo)

    w_hi = weight.astype(e4).astype(np.float32)
    w_lo = (weight - w_hi).astype(e4).astype(np.float32)
    bands = np.concatenate([_make_bands(w_hi), _make_bands(w_lo)], axis=0)
    bands8 = np.ascontiguousarray(
        bands.transpose(1, 0, 2, 3).reshape(128, 6 * 2 * 128).astype(e4)
    )

    nc = _build_program(float(bias[0]))
    in_maps = []
    for c in range(NCORES):
        blk0 = NSTRIP * c
        in_maps.append(
            {
                "xh": np.ascontiguousarray(
                    xf_hi[blk0 : blk0 + NBLK].transpose(1, 0, 2)
                ).reshape(128, NBLK * NFC),
                "xl": np.ascontiguousarray(
                    xf_lo[blk0 : blk0 + NBLK].transpose(1, 0, 2)
                ).reshape(128, NBLK * NFC),
                "bands": bands8,
            }
        )
    res = run_bass_kernel_spmd(
        nc,
        in_maps,
        list(range(NCORES)),
        trace=bool(os.environ.get("CONV_TRACE")),
    )
    LAST_RESULT = res

    full = np.empty((NCORES * NSTRIP * RB, NFO * F), np.float32)
    for c in range(NCORES):
        o = np.asarray(res.results[c]["out"]).astype(np.float32)
        full[512 * c : 512 * (c + 1)] = (
            o.reshape(NSTRIP, RB, F, NFO)
            .transpose(0, 1, 3, 2)
            .reshape(NSTRIP * RB, NFO * F)
        )
    return np.ascontiguousarray(full[:OUT, :OUT]).astype(np.float32)
